# revision 1
# baseline (speedup 1.0000x reference)
import sys
import numpy as np

sys.path.insert(0, "/root/shadow")
try:
    import setup_ntff  # noqa: F401  (registers NTFF hook; optional)
except Exception:
    pass
sys.path.insert(0, "/opt/trn_rl_repo")

import concourse.bass as bass
import concourse.bacc as bacc
import concourse.mybir as mybir
import concourse.tile as tile
from concourse.bass_utils import run_bass_kernel_spmd

N = 50000
E = 800000
M = 4
H = 4
C = 32
IN = 256
D = 128
NCORES = 8
CORE_N = 6272            # 49 blocks of 128 (padded ownership range)
NB = 49                  # blocks per core
NT = 50176               # padded node count (392 tiles of 128)
NTILES = NT // 128       # 392
TCOLS = 160              # table row: h(128) | aj(16) | ai(16)
F32 = mybir.dt.float32
I32 = mybir.dt.int32

_CACHE = {}


def _build(TPB):
    nc = bacc.Bacc("TRN2", target_bir_lowering=False, debug=False)
    AF = mybir.ActivationFunctionType
    OP = mybir.AluOpType
    ds = bass.ds

    featsT = nc.dram_tensor("featsT", [IN, NT], F32, kind="ExternalInput")
    W_in = nc.dram_tensor("W_in", [IN, D], F32, kind="ExternalInput")
    Acat = nc.dram_tensor("Acat", [D, 32], F32, kind="ExternalInput")
    iota_r = nc.dram_tensor("iota_r", [1, 128], F32, kind="ExternalInput")
    iota_c = nc.dram_tensor("iota_c", [128, 1], F32, kind="ExternalInput")
    ident = nc.dram_tensor("ident", [128, 128], F32, kind="ExternalInput")
    rel_l = nc.dram_tensor("rel_l", [1, 128], F32, kind="ExternalInput")
    rel_r = nc.dram_tensor("rel_r", [1, 640], F32, kind="ExternalInput")
    SRC = nc.dram_tensor("SRC", [128, M * NB * TPB], I32, kind="ExternalInput")
    DSTLC = nc.dram_tensor("DSTLC", [128, M * NB * TPB], F32, kind="ExternalInput")
    DSTLR = nc.dram_tensor("DSTLR", [M * NB * TPB, 128], F32, kind="ExternalInput")
    BLKIDS = nc.dram_tensor("BLKIDS", [128, NB], I32, kind="ExternalInput")

    T = nc.dram_tensor("Ttbl", [NT, TCOLS], F32)
    ERAW = nc.dram_tensor("ERAW", [M * NB * 128, 132], F32)
    OUT = nc.dram_tensor("OUT", [CORE_N, 128], F32, kind="ExternalOutput")

    with tile.TileContext(nc) as tc:
        with tc.tile_pool(name="const", bufs=1) as cp:
            W0 = cp.tile([128, 128], F32)
            nc.sync.dma_start(out=W0[:], in_=W_in[0:128, :])
            W1 = cp.tile([128, 128], F32)
            nc.sync.dma_start(out=W1[:], in_=W_in[128:256, :])
            Ac = cp.tile([128, 32], F32)
            nc.sync.dma_start(out=Ac[:], in_=Acat[:])
            io_r = cp.tile([128, 128], F32)
            nc.sync.dma_start(out=io_r[:], in_=iota_r[:].to_broadcast((128, 128)))
            io_c = cp.tile([128, 1], F32)
            nc.sync.dma_start(out=io_c[:], in_=iota_c[:])
            idn = cp.tile([128, 128], F32)
            nc.sync.dma_start(out=idn[:], in_=ident[:])
            rlr = cp.tile([128, 128], F32)
            nc.sync.dma_start(out=rlr[:], in_=rel_l[:].to_broadcast((128, 128)))
            rrr = cp.tile([128, 640], F32)
            nc.sync.dma_start(out=rrr[:], in_=rel_r[:].to_broadcast((128, 640)))

            # ---- stage 1: projection h = relu(feats @ W), a = hT-matmul ----
            with tc.tile_pool(name="s1", bufs=3) as p1, \
                 tc.tile_pool(name="s1p", bufs=2, space="PSUM") as pp1:
                with tc.For_i(0, NTILES) as i:
                    ft0 = p1.tile([128, 128], F32, tag="ft0")
                    nc.sync.dma_start(out=ft0[:], in_=featsT[0:128, ds(i * 128, 128)])
                    ft1 = p1.tile([128, 128], F32, tag="ft1")
                    nc.sync.dma_start(out=ft1[:], in_=featsT[128:256, ds(i * 128, 128)])
                    hp = pp1.tile([128, 128], F32, tag="hp")
                    nc.tensor.matmul(out=hp[:], lhsT=ft0[:], rhs=W0[:], start=True, stop=False)
                    nc.tensor.matmul(out=hp[:], lhsT=ft1[:], rhs=W1[:], start=False, stop=True)
                    hsb = p1.tile([128, 128], F32, tag="hsb")
                    nc.scalar.activation(hsb[:], hp[:], AF.Relu)
                    htp = pp1.tile([128, 128], F32, tag="htp")
                    nc.tensor.transpose(out=htp[:], in_=hsb[:], identity=idn[:])
                    hts = p1.tile([128, 128], F32, tag="hts")
                    nc.vector.tensor_copy(out=hts[:], in_=htp[:])
                    ap_ = pp1.tile([128, 32], F32, tag="ap_")
                    nc.tensor.matmul(out=ap_[:], lhsT=hts[:], rhs=Ac[:], start=True, stop=True)
                    asb = p1.tile([128, 32], F32, tag="asb")
                    nc.vector.tensor_copy(out=asb[:], in_=ap_[:])
                    nc.sync.dma_start(out=T[ds(i * 128, 128), 0:128], in_=hsb[:])
                    nc.sync.dma_start(out=T[ds(i * 128, 128), 128:160], in_=asb[:])

            # ---- stage 2: per-metapath edge aggregation ----
            for m in range(M):
                with tc.tile_pool(name=f"e{m}", bufs=3) as pe, \
                     tc.tile_pool(name=f"ep{m}", bufs=2, space="PSUM") as ppb, \
                     tc.tile_pool(name=f"ea{m}", bufs=4, space="PSUM") as ppa:
                    with tc.For_i(0, NB) as b:
                        cb = m * NB * TPB
                        idxs = pe.tile([128, TPB], I32, tag="idxs")
                        nc.sync.dma_start(out=idxs[:], in_=SRC[:, ds(b * TPB + cb, TPB)])
                        dstlc = pe.tile([128, TPB], F32, tag="dstlc")
                        nc.sync.dma_start(out=dstlc[:], in_=DSTLC[:, ds(b * TPB + cb, TPB)])
                        bst = pe.tile([128, 1], I32, tag="bst")
                        nc.sync.dma_start(out=bst[:], in_=BLKIDS[:, ds(b, 1)])
                        tblk = pe.tile([128, TCOLS], F32, tag="tblk")
                        nc.gpsimd.indirect_dma_start(
                            out=tblk[:], out_offset=None, in_=T[:],
                            in_offset=bass.IndirectOffsetOnAxis(ap=bst[:, 0:1], axis=0))
                        ohb = pe.tile([128, TPB * 128], F32, tag="ohb")
                        nc.vector.tensor_tensor(
                            out=ohb[:].rearrange("p (t n) -> p t n", n=128),
                            in0=dstlc[:].unsqueeze(2).to_broadcast((128, TPB, 128)),
                            in1=io_r[:].unsqueeze(1).to_broadcast((128, TPB, 128)),
                            op=OP.is_equal)
                        pblk = ppb.tile([128, 132], F32, tag="pblk")
                        for t in range(TPB):
                            g = pe.tile([128, TCOLS], F32, tag="g")
                            nc.gpsimd.indirect_dma_start(
                                out=g[:], out_offset=None, in_=T[:],
                                in_offset=bass.IndirectOffsetOnAxis(ap=idxs[:, t:t + 1], axis=0))
                            drow = pe.tile([128, 128], F32, tag="drow")
                            nc.sync.dma_start(
                                out=drow[:],
                                in_=DSTLR[ds(b * TPB + (cb + t), 1), :].to_broadcast((128, 128)))
                            ohT = pe.tile([128, 128], F32, tag="ohT")
                            nc.vector.tensor_tensor(
                                out=ohT[:], in0=io_c[:].to_broadcast((128, 128)),
                                in1=drow[:], op=OP.is_equal)
                            aip = ppa.tile([128, 4], F32, tag="aip")
                            nc.tensor.matmul(out=aip[:], lhsT=ohT[:],
                                             rhs=tblk[:, 144 + 4 * m:148 + 4 * m],
                                             start=True, stop=True)
                            lg = pe.tile([128, 4], F32, tag="lg")
                            nc.vector.tensor_tensor(out=lg[:], in0=aip[:],
                                                    in1=g[:, 128 + 4 * m:132 + 4 * m], op=OP.add)
                            t1 = pe.tile([128, 4], F32, tag="t1")
                            nc.vector.tensor_scalar_mul(out=t1[:], in0=lg[:], scalar1=0.2)
                            lr = pe.tile([128, 4], F32, tag="lr")
                            nc.vector.tensor_tensor(out=lr[:], in0=lg[:], in1=t1[:], op=OP.max)
                            s = pe.tile([128, 4], F32, tag="s")
                            nc.scalar.activation(s[:], lr[:], AF.Exp)
                            msg = pe.tile([128, 132], F32, tag="msg")
                            nc.vector.tensor_copy(out=msg[:, 128:132], in_=s[:])
                            nc.vector.tensor_tensor(
                                out=msg[:, 0:128].rearrange("p (h c) -> p h c", c=32),
                                in0=g[:, 0:128].rearrange("p (h c) -> p h c", c=32),
                                in1=s[:].unsqueeze(2).to_broadcast((128, 4, 32)),
                                op=OP.mult)
                            nc.tensor.matmul(out=pblk[:], lhsT=ohb[:, t * 128:(t + 1) * 128],
                                             rhs=msg[:], start=(t == 0), stop=(t == TPB - 1))
                        osb = pe.tile([128, 132], F32, tag="osb")
                        nc.vector.tensor_copy(out=osb[:], in_=pblk[:])
                        nc.sync.dma_start(out=ERAW[ds(b * 128 + m * NB * 128, 128), :], in_=osb[:])

            # ---- stage 3: relation attention + output ----
            with tc.tile_pool(name="b3", bufs=2) as p3:
                with tc.For_i(0, NB) as b:
                    bst2 = p3.tile([128, 1], I32, tag="bst2")
                    nc.sync.dma_start(out=bst2[:], in_=BLKIDS[:, ds(b, 1)])
                    tb = p3.tile([128, TCOLS], F32, tag="tb")
                    nc.gpsimd.indirect_dma_start(
                        out=tb[:], out_offset=None, in_=T[:],
                        in_offset=bass.IndirectOffsetOnAxis(ap=bst2[:, 0:1], axis=0))
                    ems = []
                    nes = []
                    for m in range(M):
                        em = p3.tile([128, 132], F32, tag=f"em{m}")
                        nc.sync.dma_start(out=em[:], in_=ERAW[ds(b * 128 + m * NB * 128, 128), :])
                        ems.append(em)
                    for m in range(M):
                        dn = p3.tile([128, 4], F32, tag=f"dn{m}")
                        nc.vector.tensor_scalar_add(out=dn[:], in0=ems[m][:, 128:132], scalar1=1e-6)
                        rc = p3.tile([128, 4], F32, tag=f"rc{m}")
                        nc.vector.reciprocal(out=rc[:], in_=dn[:])
                        ne = p3.tile([128, 128], F32, tag=f"ne{m}")
                        nc.vector.tensor_tensor(
                            out=ne[:].rearrange("p (h c) -> p h c", c=32),
                            in0=ems[m][:, 0:128].rearrange("p (h c) -> p h c", c=32),
                            in1=rc[:].unsqueeze(2).to_broadcast((128, 4, 32)), op=OP.mult)
                        nes.append(ne)
                    bl0 = p3.tile([128, 128], F32, tag="bl0")
                    nc.vector.tensor_tensor(out=bl0[:], in0=tb[:, 0:128], in1=rlr[:], op=OP.mult)
                    blr = p3.tile([128, 128], F32, tag="blr")
                    nc.scalar.activation(blr[:], bl0[:], AF.Relu)
                    bmat = p3.tile([128, 20], F32, tag="bmat")
                    for r in range(5):
                        er = nes[r][:] if r < 4 else tb[:, 0:128]
                        tm1 = p3.tile([128, 128], F32, tag="tm1")
                        nc.vector.tensor_tensor(out=tm1[:], in0=er, in1=rrr[:, r * 128:(r + 1) * 128], op=OP.mult)
                        tm2 = p3.tile([128, 128], F32, tag="tm2")
                        nc.scalar.activation(tm2[:], tm1[:], AF.Relu)
                        tm3 = p3.tile([128, 128], F32, tag="tm3")
                        nc.vector.tensor_tensor(out=tm3[:], in0=tm2[:], in1=blr[:], op=OP.mult)
                        nc.vector.reduce_sum(
                            out=bmat[:, r * 4:(r + 1) * 4],
                            in_=tm3[:].rearrange("p (h c) -> p h c", c=32),
                            axis=mybir.AxisListType.X)
                    vmax = p3.tile([128, 4], F32, tag="vmax")
                    bview = bmat[:].rearrange("p (r h) -> p h r", h=4)
                    nc.vector.reduce_max(out=vmax[:], in_=bview, axis=mybir.AxisListType.X)
                    eb = p3.tile([128, 20], F32, tag="eb")
                    nc.vector.tensor_tensor(
                        out=eb[:].rearrange("p (r h) -> p h r", h=4),
                        in0=bview, in1=vmax[:].unsqueeze(2).to_broadcast((128, 4, 5)),
                        op=OP.subtract)
                    eb2 = p3.tile([128, 20], F32, tag="eb2")
                    nc.scalar.activation(eb2[:], eb[:], AF.Exp)
                    vs = p3.tile([128, 4], F32, tag="vs")
                    nc.vector.reduce_sum(out=vs[:], in_=eb2[:].rearrange("p (r h) -> p h r", h=4),
                                         axis=mybir.AxisListType.X)
                    rs = p3.tile([128, 4], F32, tag="rs")
                    nc.vector.reciprocal(out=rs[:], in_=vs[:])
                    bw = p3.tile([128, 20], F32, tag="bw")
                    nc.vector.tensor_tensor(
                        out=bw[:].rearrange("p (r h) -> p h r", h=4),
                        in0=eb2[:].rearrange("p (r h) -> p h r", h=4),
                        in1=rs[:].unsqueeze(2).to_broadcast((128, 4, 5)), op=OP.mult)
                    acc = p3.tile([128, 128], F32, tag="acc")
                    for r in range(5):
                        er = nes[r][:] if r < 4 else tb[:, 0:128]
                        if r == 0:
                            nc.vector.tensor_tensor(
                                out=acc[:].rearrange("p (h c) -> p h c", c=32),
                                in0=er.rearrange("p (h c) -> p h c", c=32),
                                in1=bw[:, 0:4].unsqueeze(2).to_broadcast((128, 4, 32)), op=OP.mult)
                        else:
                            tm4 = p3.tile([128, 128], F32, tag="tm4")
                            nc.vector.tensor_tensor(
                                out=tm4[:].rearrange("p (h c) -> p h c", c=32),
                                in0=er.rearrange("p (h c) -> p h c", c=32),
                                in1=bw[:, r * 4:(r + 1) * 4].unsqueeze(2).to_broadcast((128, 4, 32)),
                                op=OP.mult)
                            nc.vector.tensor_tensor(out=acc[:], in0=acc[:], in1=tm4[:], op=OP.add)
                    ob = p3.tile([128, 128], F32, tag="ob")
                    nc.scalar.activation(ob[:], acc[:], AF.Relu)
                    nc.sync.dma_start(out=OUT[ds(b * 128, 128), :], in_=ob[:])

    nc.compile()
    return nc


def _prep_host(feats, edge_index, W, attn, rel_attn_l, rel_attn_r):
    featsT = np.zeros((IN, NT), dtype=np.float32)
    featsT[:, :N] = np.asarray(feats, dtype=np.float32).T
    Acat = np.zeros((D, 32), dtype=np.float32)
    attn = np.asarray(attn, dtype=np.float32)
    for m in range(M):
        for h in range(H):
            Acat[h * C:(h + 1) * C, m * H + h] = attn[m, h, C:]        # aj (src side)
            Acat[h * C:(h + 1) * C, 16 + m * H + h] = attn[m, h, :C]   # ai (dst side)
    iota_r = np.arange(128, dtype=np.float32).reshape(1, 128)
    iota_c = np.arange(128, dtype=np.float32).reshape(128, 1)
    ident = np.eye(128, dtype=np.float32)
    rel_l = np.asarray(rel_attn_l, dtype=np.float32).reshape(1, 128)
    rel_r = np.asarray(rel_attn_r, dtype=np.float32).reshape(1, 640)

    ei = np.asarray(edge_index)
    percore = []
    TPB = 1
    for k in range(NCORES):
        cm = []
        for m in range(M):
            src = ei[m, 0]
            dst = ei[m, 1]
            sel = (dst // CORE_N) == k
            ls = (dst[sel] - CORE_N * k).astype(np.int64)
            sr = src[sel].astype(np.int64)
            order = np.argsort(ls, kind="stable")
            ls = ls[order]; sr = sr[order]
            bid = ls // 128
            first = np.searchsorted(bid, bid)
            rank = np.arange(len(ls)) - first
            cnt = np.bincount(bid.astype(np.int64), minlength=NB)
            TPB = max(TPB, int(np.ceil(cnt.max() / 128)))
            cm.append((ls, sr, bid, rank))
        percore.append(cm)

    in_maps = []
    for k in range(NCORES):
        SRCa = np.zeros((128, M * NB * TPB), dtype=np.int32)
        DSTLCa = np.full((128, M * NB * TPB), 999.0, dtype=np.float32)
        DSTLRa = np.full((M * NB * TPB, 128), 999.0, dtype=np.float32)
        for m in range(M):
            ls, sr, bid, rank = percore[k][m]
            t = rank // 128
            p = rank % 128
            col = (m * NB + bid) * TPB + t
            SRCa[p, col] = sr
            dl = (ls - bid * 128).astype(np.float32)
            DSTLCa[p, col] = dl
            DSTLRa[col, p] = dl
        blkids = np.minimum(
            CORE_N * k + np.arange(NB)[None, :] * 128 + np.arange(128)[:, None],
            N - 1).astype(np.int32)
        in_maps.append({
            "featsT": featsT, "W_in": np.asarray(W, dtype=np.float32),
            "Acat": Acat, "iota_r": iota_r, "iota_c": iota_c, "ident": ident,
            "rel_l": rel_l, "rel_r": rel_r,
            "SRC": SRCa, "DSTLC": DSTLCa, "DSTLR": DSTLRa, "BLKIDS": blkids,
        })
    return TPB, in_maps


def kernel(feats, edge_index, W, b, attn, rel_attn_l, rel_attn_r, rel_attn_bias,
           _trace=False):
    TPB, in_maps = _prep_host(feats, edge_index, W, attn, rel_attn_l, rel_attn_r)
    if TPB not in _CACHE:
        _CACHE[TPB] = _build(TPB)
    nc = _CACHE[TPB]
    res = run_bass_kernel_spmd(nc, in_maps, core_ids=list(range(NCORES)),
                               trace=_trace)
    parts = []
    for k in range(NCORES):
        rows = min(CORE_N, N - CORE_N * k)
        parts.append(res.results[k]["OUT"][:rows])
    out = np.concatenate(parts, axis=0).astype(np.float32)
    if _trace:
        kernel._last_exec_ns = res.exec_time_ns
    return out



# revision 17
# speedup vs baseline: 1.5197x; 1.5197x over previous
import sys
import numpy as np

sys.path.insert(0, "/root/shadow")
try:
    import setup_ntff  # noqa: F401  (registers NTFF hook; optional)
except Exception:
    pass
sys.path.insert(0, "/opt/trn_rl_repo")

import ml_dtypes
import concourse.bass as bass
import concourse.bacc as bacc
import concourse.mybir as mybir
import concourse.tile as tile
from concourse import library_config
from concourse.bass_utils import run_bass_kernel_spmd

N = 50000
E = 800000
M = 4
H = 4
C = 32
IN = 256
D = 128
NCORES = 8
CORE_N = 6272            # 49 blocks of 128 per core
NB = 49
NT = 50176               # padded node count (392 tiles of 128)
NTILES = 392
LO_TILES = 256           # first 256 tiles (32768 rows) go to T_LO
RLO = LO_TILES * 128     # 32768
RHI = NT - RLO           # 17408
TC = 256                 # table row cols (bf16): h(128)|aj(16)|ai(16)|pad(96)
F32 = mybir.dt.float32
BF16 = mybir.dt.bfloat16
I32 = mybir.dt.int32
I16 = mybir.dt.int16
BIGIDX = 1 << 28         # sentinel for masked tblk gathers

_CACHE = {}


def _build(TL, TH):
    NCH = M * (TL + TH)          # edge chunks per block
    NPC = NCH * 128 // 512       # 512-col broadcast pieces per block
    assert NCH * 128 % 512 == 0
    nc = bacc.Bacc("TRN2", target_bir_lowering=False, debug=False)
    AF = mybir.ActivationFunctionType
    OP = mybir.AluOpType
    ds = bass.ds

    featsT = nc.dram_tensor("featsT", [IN, NT], BF16, kind="ExternalInput")
    W_in = nc.dram_tensor("W_in", [IN, D], BF16, kind="ExternalInput")
    Acat = nc.dram_tensor("Acat", [D, 32], BF16, kind="ExternalInput")
    iota_r = nc.dram_tensor("iota_r", [1, 128], BF16, kind="ExternalInput")
    iota_c = nc.dram_tensor("iota_c", [128, 1], F32, kind="ExternalInput")
    ident = nc.dram_tensor("ident", [128, 128], BF16, kind="ExternalInput")
    ones1 = nc.dram_tensor("ones1", [1, 128], BF16, kind="ExternalInput")
    rel_l = nc.dram_tensor("rel_l", [1, 128], BF16, kind="ExternalInput")
    rel_r = nc.dram_tensor("rel_r", [1, 640], BF16, kind="ExternalInput")
    IDXLO = nc.dram_tensor("IDXLO", [128, NB * M * TL * 8], I16,
                           kind="ExternalInput")
    IDXHI = nc.dram_tensor("IDXHI", [128, NB * M * TH * 8], I16,
                           kind="ExternalInput")
    DSTLC = nc.dram_tensor("DSTLC", [128, NB * NCH], BF16,
                           kind="ExternalInput")
    DSTROW = nc.dram_tensor("DSTROW", [NB, NCH * 128], BF16,
                            kind="ExternalInput")
    BLKLO = nc.dram_tensor("BLKLO", [128, NB], I32, kind="ExternalInput")

    T_LO = nc.dram_tensor("T_LO", [RLO, TC], BF16)
    T_HI = nc.dram_tensor("T_HI", [RHI, TC], BF16)
    TFULL = nc.dram_tensor("TFULL", [NT, 160], BF16)
    OUT = nc.dram_tensor("OUT", [CORE_N, 128], F32, kind="ExternalOutput")

    # chunk -> metapath map
    def ch_m(ch):
        return ch // TL if ch < M * TL else (ch - M * TL) // TH

    # chunk runs per metapath: [(start_chunk, n_chunks), ...]
    def m_runs(m):
        return [(m * TL, TL), (M * TL + m * TH, TH)]

    with tile.TileContext(nc) as tc:
        with tc.tile_pool(name="const", bufs=1) as cp:
            nc.gpsimd.load_library(library_config.mlp)
            W0 = cp.tile([128, 128], BF16)
            nc.sync.dma_start(out=W0[:], in_=W_in[0:128, :])
            W1 = cp.tile([128, 128], BF16)
            nc.sync.dma_start(out=W1[:], in_=W_in[128:256, :])
            Ac = cp.tile([128, 32], BF16)
            nc.sync.dma_start(out=Ac[:], in_=Acat[:])
            io_r = cp.tile([128, 128], BF16)
            nc.sync.dma_start(out=io_r[:], in_=iota_r[:].to_broadcast((128, 128)))
            io_c = cp.tile([128, 1], F32)
            nc.sync.dma_start(out=io_c[:], in_=iota_c[:])
            idn = cp.tile([128, 128], BF16)
            nc.sync.dma_start(out=idn[:], in_=ident[:])
            on1 = cp.tile([1, 128], BF16)
            nc.sync.dma_start(out=on1[:], in_=ones1[:])
            rlr = cp.tile([128, 128], BF16)
            nc.sync.dma_start(out=rlr[:], in_=rel_l[:].to_broadcast((128, 128)))
            rrr = cp.tile([128, 640], BF16)
            nc.sync.dma_start(out=rrr[:], in_=rel_r[:].to_broadcast((128, 640)))

            # zero-fill the pad columns of the tables (gathers read full
            # 512B rows; sim flags uninitialized DRAM)
            zpad = cp.tile([128, TC - 160], BF16)
            nc.gpsimd.memset(zpad[:], 0.0)
            nc.sync.dma_start(
                out=T_LO[:, 160:TC].rearrange("(t p) c -> p t c", p=128),
                in_=zpad[:].unsqueeze(1).to_broadcast(
                    (128, RLO // 128, TC - 160)))
            nc.sync.dma_start(
                out=T_HI[:, 160:TC].rearrange("(t p) c -> p t c", p=128),
                in_=zpad[:].unsqueeze(1).to_broadcast(
                    (128, RHI // 128, TC - 160)))

            # ---- stage 1: h = relu(feats @ W); a = hT @ Acat; write tables
            def proj(lo, hi, dst, tag):
                with tc.tile_pool(name=f"s1{tag}", bufs=3) as p1, \
                     tc.tile_pool(name=f"s1p{tag}", bufs=2, space="PSUM") as pp1:
                    with tc.For_i(lo, hi) as i:
                        ft0 = p1.tile([128, 128], BF16, tag="ft0")
                        nc.sync.dma_start(out=ft0[:],
                                          in_=featsT[0:128, ds(i * 128, 128)])
                        ft1 = p1.tile([128, 128], BF16, tag="ft1")
                        nc.sync.dma_start(out=ft1[:],
                                          in_=featsT[128:256, ds(i * 128, 128)])
                        hp = pp1.tile([128, 128], F32, tag="hp")
                        nc.tensor.matmul(out=hp[:], lhsT=ft0[:], rhs=W0[:],
                                         start=True, stop=False)
                        nc.tensor.matmul(out=hp[:], lhsT=ft1[:], rhs=W1[:],
                                         start=False, stop=True)
                        hsb = p1.tile([128, 128], BF16, tag="hsb")
                        nc.vector.tensor_scalar_max(out=hsb[:], in0=hp[:],
                                                    scalar1=0.0)
                        htp = pp1.tile([128, 128], BF16, tag="htp")
                        nc.tensor.transpose(out=htp[:], in_=hsb[:],
                                            identity=idn[:])
                        hts = p1.tile([128, 128], BF16, tag="hts")
                        nc.vector.tensor_copy(out=hts[:], in_=htp[:])
                        ap_ = pp1.tile([128, 32], F32, tag="ap_")
                        nc.tensor.matmul(out=ap_[:], lhsT=hts[:], rhs=Ac[:],
                                         start=True, stop=True)
                        asb = p1.tile([128, 32], BF16, tag="asb")
                        nc.vector.tensor_copy(out=asb[:], in_=ap_[:])
                        base = ds(i * 128 - lo * 128, 128)
                        nc.sync.dma_start(out=dst[base, 0:128], in_=hsb[:])
                        nc.sync.dma_start(out=dst[base, 128:160], in_=asb[:])
                        gbase = ds(i * 128, 128)
                        nc.sync.dma_start(out=TFULL[gbase, 0:128], in_=hsb[:])
                        nc.sync.dma_start(out=TFULL[gbase, 128:160],
                                          in_=asb[:])

            proj(0, LO_TILES, T_LO, "a")
            proj(LO_TILES, NTILES, T_HI, "b")

            # ---- stage 2+3 fused block loop
            with tc.tile_pool(name="eb", bufs=2) as pe, \
                 tc.tile_pool(name="ohbp", bufs=2) as pb, \
                 tc.tile_pool(name="wk", bufs=1) as pw, \
                 tc.tile_pool(name="oh2p", bufs=4) as po, \
                 tc.tile_pool(name="s3", bufs=1) as p3, \
                 tc.tile_pool(name="ppb", bufs=1, space="PSUM") as ppb, \
                 tc.tile_pool(name="ppc", bufs=2, space="PSUM") as ppc:
                with tc.For_i(0, NB) as b:
                    # block's own rows (h + ai)
                    blo = pe.tile([128, 1], I32, tag="blo")
                    nc.sync.dma_start(out=blo[:], in_=BLKLO[:, ds(b, 1)])
                    tblk = pe.tile([128, 160], BF16, tag="tblk")
                    nc.gpsimd.indirect_dma_start(
                        out=tblk[:], out_offset=None, in_=TFULL[:],
                        in_offset=bass.IndirectOffsetOnAxis(ap=blo[:, 0:1],
                                                            axis=0))

                    # edge gathers (lo+hi tables)
                    ixl = pe.tile([128, M * TL * 8], I16, tag="ixl")
                    nc.sync.dma_start(out=ixl[:],
                                      in_=IDXLO[:, ds(b * M * TL * 8,
                                                      M * TL * 8)])
                    ixh = pe.tile([128, M * TH * 8], I16, tag="ixh")
                    nc.sync.dma_start(out=ixh[:],
                                      in_=IDXHI[:, ds(b * M * TH * 8,
                                                      M * TH * 8)])
                    G = pe.tile([128, NCH * TC], BF16, tag="G")
                    for m in range(M):
                        nc.gpsimd.dma_gather(
                            out_ap=G[:, m * TL * TC:(m + 1) * TL * TC]
                                .rearrange("p (j e) -> p j e", e=TC),
                            in_ap=T_LO[:],
                            idxs_ap=ixl[:, ds(m * TL * 8, TL * 8)],
                            num_idxs=TL * 128, num_idxs_reg=TL * 128,
                            elem_size=TC, single_packet=False)
                        nc.gpsimd.dma_gather(
                            out_ap=G[:, (M * TL + m * TH) * TC:
                                     (M * TL + (m + 1) * TH) * TC]
                                .rearrange("p (j e) -> p j e", e=TC),
                            in_ap=T_HI[:],
                            idxs_ap=ixh[:, ds(m * TH * 8, TH * 8)],
                            num_idxs=TH * 128, num_idxs_reg=TH * 128,
                            elem_size=TC, single_packet=False)

                    # one-hot (edge-partition orientation): ohb[p,(ch,n)]
                    dstlc = pe.tile([128, NCH], BF16, tag="dstlc")
                    nc.sync.dma_start(out=dstlc[:],
                                      in_=DSTLC[:, ds(b * NCH, NCH)])
                    ohb = pb.tile([128, NCH * 128], BF16, tag="ohb")
                    nc.vector.tensor_tensor(
                        out=ohb[:].rearrange("p (t n) -> p t n", n=128),
                        in0=dstlc[:].unsqueeze(2).to_broadcast((128, NCH, 128)),
                        in1=io_r[:].unsqueeze(1).to_broadcast((128, NCH, 128)),
                        op=OP.is_equal)

                    # dst-local per edge, broadcast via PE; oh2 pieces + aip
                    slab = pe.tile([1, NCH * 128], BF16, tag="slab")
                    nc.sync.dma_start(out=slab[:], in_=DSTROW[ds(b, 1), :])
                    aip = ppb.tile([128, NCH * 4], F32, tag="aip")
                    for pc in range(NPC):
                        bcp = ppc.tile([128, 512], F32, tag="bcp")
                        nc.tensor.matmul(out=bcp[:], lhsT=on1[:],
                                         rhs=slab[0:1, ds(pc * 512, 512)],
                                         start=True, stop=True)
                        oh2p = po.tile([128, 512], BF16, tag="oh2p")
                        nc.vector.tensor_tensor(
                            out=oh2p[:],
                            in0=io_c[:].to_broadcast((128, 512)),
                            in1=bcp[:], op=OP.is_equal)
                        for q in range(4):
                            ch = pc * 4 + q
                            ac = 144 + 4 * ch_m(ch)
                            nc.tensor.matmul(
                                out=aip[:, ds(ch * 4, 4)],
                                lhsT=oh2p[:, ds(q * 128, 128)],
                                rhs=tblk[:, ac:ac + 4],
                                start=True, stop=True)

                    # logits -> alpha weights
                    lg = pw.tile([128, NCH * 4], F32, tag="lg")
                    Gv = G[:].rearrange("p (t w) -> p t w", w=TC)
                    for m in range(M):
                        for c0, ncnk in m_runs(m):
                            nc.vector.tensor_tensor(
                                out=lg[:].rearrange("p (t w) -> p t w", w=4)
                                    [:, c0:c0 + ncnk, :],
                                in0=aip[:].rearrange("p (t w) -> p t w", w=4)
                                    [:, c0:c0 + ncnk, :],
                                in1=Gv[:, c0:c0 + ncnk,
                                       128 + 4 * m:132 + 4 * m],
                                op=OP.add)
                    t1 = pw.tile([128, NCH * 4], F32, tag="t1")
                    nc.vector.tensor_scalar_mul(out=t1[:], in0=lg[:],
                                                scalar1=0.2)
                    lr = pw.tile([128, NCH * 4], F32, tag="lr")
                    nc.vector.tensor_tensor(out=lr[:], in0=lg[:], in1=t1[:],
                                            op=OP.max)
                    s = pw.tile([128, NCH * 4], BF16, tag="s")
                    nc.scalar.activation(s[:], lr[:], AF.Exp)

                    # messages
                    msg = pw.tile([128, NCH * 132], BF16, tag="msg")
                    nc.vector.tensor_tensor(
                        out=msg[:].rearrange("p (t w) -> p t w", w=132)
                            [:, :, 0:128].rearrange("p t (h c) -> p t h c",
                                                    c=32),
                        in0=Gv[:, :, 0:128].rearrange("p t (h c) -> p t h c",
                                                      c=32),
                        in1=s[:].rearrange("p (t h) -> p t h", h=4)
                            .unsqueeze(3).to_broadcast((128, NCH, 4, 32)),
                        op=OP.mult)
                    nc.vector.tensor_copy(
                        out=msg[:].rearrange("p (t w) -> p t w", w=132)
                            [:, :, 128:132],
                        in_=s[:].rearrange("p (t h) -> p t h", h=4))

                    # scatter-add per metapath via PE
                    pbs = []
                    for m in range(M):
                        pblk = ppb.tile([128, 132], F32, tag=f"pblk{m}")
                        chs = [c0 + j for c0, ncnk in m_runs(m)
                               for j in range(ncnk)]
                        for k, ch in enumerate(chs):
                            nc.tensor.matmul(
                                out=pblk[:],
                                lhsT=ohb[:, ds(ch * 128, 128)],
                                rhs=msg[:, ds(ch * 132, 132)],
                                start=(k == 0), stop=(k == len(chs) - 1))
                        pbs.append(pblk)

                    # ---- stage 3: relation attention (fused per block)
                    nes = p3.tile([128, 640], BF16, tag="nes")
                    for m in range(M):
                        dn = p3.tile([128, 4], F32, tag=f"dn{m}")
                        nc.vector.tensor_scalar_add(out=dn[:],
                                                    in0=pbs[m][:, 128:132],
                                                    scalar1=1e-6)
                        rc = p3.tile([128, 4], F32, tag=f"rc{m}")
                        nc.vector.reciprocal(out=rc[:], in_=dn[:])
                        nc.vector.tensor_tensor(
                            out=nes[:, ds(m * 128, 128)].rearrange(
                                "p (h c) -> p h c", c=32),
                            in0=pbs[m][:, 0:128].rearrange(
                                "p (h c) -> p h c", c=32),
                            in1=rc[:].unsqueeze(2).to_broadcast((128, 4, 32)),
                            op=OP.mult)
                    nc.vector.tensor_copy(out=nes[:, 512:640],
                                          in_=tblk[:, 0:128])
                    bl0 = p3.tile([128, 128], F32, tag="bl0")
                    nc.vector.tensor_tensor(out=bl0[:], in0=tblk[:, 0:128],
                                            in1=rlr[:], op=OP.mult)
                    blr = p3.tile([128, 128], BF16, tag="blr")
                    nc.vector.tensor_scalar_max(out=blr[:], in0=bl0[:],
                                                scalar1=0.0)
                    tm = p3.tile([128, 640], F32, tag="tm")
                    nc.vector.tensor_tensor(out=tm[:], in0=nes[:], in1=rrr[:],
                                            op=OP.mult)
                    tm2 = p3.tile([128, 640], BF16, tag="tm2")
                    nc.vector.tensor_scalar_max(out=tm2[:], in0=tm[:],
                                                scalar1=0.0)
                    tm3 = p3.tile([128, 640], BF16, tag="tm3")
                    nc.vector.tensor_tensor(
                        out=tm3[:].rearrange("p (r k) -> p r k", k=128),
                        in0=tm2[:].rearrange("p (r k) -> p r k", k=128),
                        in1=blr[:].unsqueeze(1).to_broadcast((128, 5, 128)),
                        op=OP.mult)
                    bmat = p3.tile([128, 20], F32, tag="bmat")
                    nc.vector.reduce_sum(
                        out=bmat[:],
                        in_=tm3[:].rearrange("p (g c) -> p g c", c=32),
                        axis=mybir.AxisListType.X)
                    bview = bmat[:].rearrange("p (r h) -> p h r", h=4)
                    vmax = p3.tile([128, 4], F32, tag="vmax")
                    nc.vector.reduce_max(out=vmax[:], in_=bview,
                                         axis=mybir.AxisListType.X)
                    eb = p3.tile([128, 20], F32, tag="ebt")
                    nc.vector.tensor_tensor(
                        out=eb[:].rearrange("p (r h) -> p h r", h=4),
                        in0=bview,
                        in1=vmax[:].unsqueeze(2).to_broadcast((128, 4, 5)),
                        op=OP.subtract)
                    eb2 = p3.tile([128, 20], F32, tag="eb2")
                    nc.scalar.activation(eb2[:], eb[:], AF.Exp)
                    vs = p3.tile([128, 4], F32, tag="vs")
                    nc.vector.reduce_sum(
                        out=vs[:],
                        in_=eb2[:].rearrange("p (r h) -> p h r", h=4),
                        axis=mybir.AxisListType.X)
                    rs = p3.tile([128, 4], F32, tag="rs")
                    nc.vector.reciprocal(out=rs[:], in_=vs[:])
                    bw = p3.tile([128, 20], BF16, tag="bw")
                    nc.vector.tensor_tensor(
                        out=bw[:].rearrange("p (r h) -> p h r", h=4),
                        in0=eb2[:].rearrange("p (r h) -> p h r", h=4),
                        in1=rs[:].unsqueeze(2).to_broadcast((128, 4, 5)),
                        op=OP.mult)
                    wsum = p3.tile([128, 640], F32, tag="wsum")
                    nc.vector.tensor_tensor(
                        out=wsum[:].rearrange("p (r h c) -> p r h c",
                                              h=4, c=32),
                        in0=nes[:].rearrange("p (r h c) -> p r h c",
                                             h=4, c=32),
                        in1=bw[:].rearrange("p (r h) -> p r h", h=4)
                            .unsqueeze(3).to_broadcast((128, 5, 4, 32)),
                        op=OP.mult)
                    acc = p3.tile([128, 128], F32, tag="acc")
                    nc.vector.reduce_sum(
                        out=acc[:],
                        in_=wsum[:].rearrange("p (r k) -> p k r", k=128),
                        axis=mybir.AxisListType.X)
                    ob = p3.tile([128, 128], F32, tag="ob")
                    nc.vector.tensor_scalar_max(out=ob[:], in0=acc[:],
                                                scalar1=0.0)
                    nc.sync.dma_start(out=OUT[ds(b * 128, 128), :], in_=ob[:])

    nc.compile()
    return nc


def _pack_idx(flat):
    # flat [num] int -> [128, num//16] int16 (16-partition wrap, tiled x8)
    num = len(flat)
    a = np.asarray(flat, np.int16).reshape(num // 16, 16).T
    return np.tile(a, (8, 1))


def _prep_host(feats, edge_index, W, attn, rel_attn_l, rel_attn_r):
    featsT = np.zeros((IN, NT), dtype=ml_dtypes.bfloat16)
    featsT[:, :N] = np.asarray(feats, dtype=np.float32).T.astype(
        ml_dtypes.bfloat16)
    Acat = np.zeros((D, 32), dtype=np.float32)
    attn = np.asarray(attn, dtype=np.float32)
    for m in range(M):
        for h in range(H):
            Acat[h * C:(h + 1) * C, m * H + h] = attn[m, h, C:]       # aj
            Acat[h * C:(h + 1) * C, 16 + m * H + h] = attn[m, h, :C]  # ai
    consts = {
        "W_in": np.asarray(W, np.float32).astype(ml_dtypes.bfloat16),
        "Acat": Acat.astype(ml_dtypes.bfloat16),
        "iota_r": np.arange(128, dtype=np.float32).reshape(1, 128).astype(
            ml_dtypes.bfloat16),
        "iota_c": np.arange(128, dtype=np.float32).reshape(128, 1),
        "ident": np.eye(128, dtype=np.float32).astype(ml_dtypes.bfloat16),
        "ones1": np.ones((1, 128), np.float32).astype(ml_dtypes.bfloat16),
        "rel_l": np.asarray(rel_attn_l, np.float32).reshape(1, 128).astype(
            ml_dtypes.bfloat16),
        "rel_r": np.asarray(rel_attn_r, np.float32).reshape(1, 640).astype(
            ml_dtypes.bfloat16),
        "featsT": featsT,
    }

    ei = np.asarray(edge_index)
    percore = []
    TL = TH = 1
    for k in range(NCORES):
        cm = []
        for m in range(M):
            src = ei[m, 0].astype(np.int64)
            dst = ei[m, 1].astype(np.int64)
            sel = (dst // CORE_N) == k
            ls = dst[sel] - CORE_N * k
            sr = src[sel]
            bid = ls // 128
            per_b = []
            for b in range(NB):
                eb = bid == b
                lsb = ls[eb] - b * 128
                srb = sr[eb]
                lo = srb < RLO
                per_b.append((lsb[lo], srb[lo], lsb[~lo], srb[~lo] - RLO))
                TL = max(TL, (len(srb[lo]) + 127) // 128)
                TH = max(TH, (int((~lo).sum()) + 127) // 128)
            cm.append(per_b)
        percore.append(cm)

    NCH = M * (TL + TH)
    in_maps = []
    for k in range(NCORES):
        IDXLO = np.zeros((NB, M * TL * 128), np.int16)
        IDXHI = np.zeros((NB, M * TH * 128), np.int16)
        DSTLCa = np.full((128, NB * NCH), 999.0, np.float32)
        DSTROWa = np.full((NB, NCH * 128), 999.0, np.float32)
        for b in range(NB):
            for m in range(M):
                llo, slo, lhi, shi = percore[k][m][b]
                for (ll, ss, base, idxarr) in (
                        (llo, slo, m * TL, IDXLO),
                        (lhi, shi, M * TL + m * TH, IDXHI)):
                    nn = len(ll)
                    j = np.arange(nn)
                    ch = (base + j // 128) if idxarr is IDXLO else \
                         (m * TH + j // 128)
                    # chunk index local to this idx array
                    chl = (m * TL + j // 128) if idxarr is IDXLO else \
                          (m * TH + j // 128)
                    p = j % 128
                    idxarr[b, chl * 128 + p] = ss
                    gch = chl if idxarr is IDXLO else M * TL + chl
                    DSTLCa[p, b * NCH + gch] = ll
                    DSTROWa[b, gch * 128 + p] = ll
        blo = np.minimum(CORE_N * k + np.arange(NB)[None, :] * 128
                         + np.arange(128)[:, None], N - 1).astype(np.int32)
        im = dict(consts)
        im.update({
            "IDXLO": np.concatenate([_pack_idx(IDXLO[b]) for b in range(NB)],
                                    axis=1),
            "IDXHI": np.concatenate([_pack_idx(IDXHI[b]) for b in range(NB)],
                                    axis=1),
            "DSTLC": DSTLCa.astype(ml_dtypes.bfloat16),
            "DSTROW": DSTROWa.astype(ml_dtypes.bfloat16),
            "BLKLO": blo,
        })
        in_maps.append(im)
    return TL, TH, in_maps


def kernel(feats, edge_index, W, b, attn, rel_attn_l, rel_attn_r,
           rel_attn_bias, _trace=False):
    TL, TH, in_maps = _prep_host(feats, edge_index, W, attn,
                                 rel_attn_l, rel_attn_r)
    if (TL, TH) not in _CACHE:
        _CACHE[(TL, TH)] = _build(TL, TH)
    nc = _CACHE[(TL, TH)]
    res = run_bass_kernel_spmd(nc, in_maps, core_ids=list(range(NCORES)),
                               trace=_trace)
    parts = []
    for k in range(NCORES):
        rows = min(CORE_N, N - CORE_N * k)
        parts.append(res.results[k]["OUT"][:rows])
    out = np.concatenate(parts, axis=0).astype(np.float32)
    if _trace:
        kernel._last_exec_ns = res.exec_time_ns
    return out
